# revision 1
# baseline (speedup 1.0000x reference)
"""Trainium2 Bass kernel for nn_AttentionStem (sparse local attention stem).

Math per output element (b, c, h, w), window kk = (di, dj) in 4x4, PAD=2:
  E[c,h,w]   = (emb_a[c,w] + emb_b[c,h]) * emb_mix[c,h,w]
  e1_kk      = exp(v_kk^2 * E)                  (softmax-1 numerator)
  q'         = q / sum_kk(e1)                   (fold softmax-1 denom into q)
  e2_kk      = exp(q' * k_kk * e1_kk)           (softmax-2 numerator)
  out        = sum_kk(e2 * v_kk) / sum_kk(e2)

Sharding: pure data parallel, one batch element per NeuronCore (8 cores).
Layout per core: SBUF partition p = 64*half + c  (half = h<64 ? 0 : 1),
free dims stream (h, w); KK tiles are [128, KK, n] with kk outermost.
The three sum_kk reductions run on the TensorEngine as chains of 16
PSUM-accumulating identity matmuls (exact fp32 sums, same layout out).
16-bit tensors use fp16 where the value range allows and bf16 where the
unnormalized exp(t1) magnitudes (~e^56) require the wider exponent; DVE
instructions are split per (di, dj-parity) so both operands of every
tensor_tensor keep 4-byte alignment, which the DVE 2x packed mode needs.
"""
import sys, os
for _p in ("/opt/trn_rl_repo", "/root/.axon_site/_ro/trn_rl_repo"):
    if os.path.isdir(_p) and _p not in sys.path:
        sys.path.insert(0, _p)

from contextlib import ExitStack, nullcontext as _nullcm
import numpy as np

import concourse.bass as bass
import concourse.bacc as bacc
import concourse.tile as tile
from concourse import mybir
import concourse.bass_utils as bass_utils
from concourse.bass_types import AP
from concourse import masks

N_CORES = 8
B, CIN, H, W = 8, 3, 128, 128
C = 64
K, PAD, KK = 4, 2, 16
HP, WP = H + 2 * PAD, W + 2 * PAD  # 132, 132
HH = H // 2                        # rows per half (64)

F32 = mybir.dt.float32
BF16 = mybir.dt.bfloat16
F16 = mybir.dt.float16
F32R = mybir.dt.float32r
MULT = mybir.AluOpType.mult
ADD = mybir.AluOpType.add
EXP = mybir.ActivationFunctionType.Exp
SQUARE = mybir.ActivationFunctionType.Square

CH = 2  # h-rows per half per chunk

# Precision / engine configuration.
#   kk:    dtype of the KK-expanded pipeline (maps, t1/e1/m1/s2/e2/m2)
#   conv:  dtype of the 1x1-conv matmuls (fp32: 4 cyc/row, fp32r/bf16: 1)
#   e2_fp32: keep softmax-2 numerators in fp32 (accuracy of the output path)
# Per-tensor dtypes of the KK pipeline. fp16 where the value range allows
# (8x finer mantissa than bf16); bf16 where unnormalized exp(t1) magnitudes
# (up to ~e^56) must be representable (e1, m1, and the q/sum(e1) scale qp).
DTS_F16 = dict(map=F16, E=F16, t1=F16, e1=BF16, qp=BF16, m1=F16, s2=F16,
               e2=F16, m2=F16, mix=F16)
DTS_BF16 = {k: BF16 for k in DTS_F16}
DTS_F32 = {k: F32 for k in DTS_F16}
CFG = dict(kk=BF16, dts=DTS_F16, conv=F32R, e2_fp32=False, mh=8, pool_tt=())


def _ap(base: AP, offset: int, dims):
    """Build a custom free-dim AP on a tile/dram AP, keeping its partition dim."""
    return AP(tensor=base.tensor, offset=base.offset + offset,
              ap=[list(base.ap[0])] + [list(d) for d in dims])


def build_kernel(nc, ch: int = CH, cfg=None, reps: int = 0):
    """reps>0 wraps the whole body in a hardware loop (for benchmarking)."""
    cfg = dict(CFG if cfg is None else cfg)
    f32 = F32
    dts = dict(cfg.get("dts") or {k: cfg["kk"] for k in DTS_F16})
    d_e2 = f32 if cfg["e2_fp32"] else dts["e2"]
    dkk = dts["t1"]
    dcv = cfg["conv"]               # conv matmul dtype
    split = dts["t1"] != f32       # parity-split DVE instrs for 2x mode
    pool_tt = set(cfg.get("pool_tt") or ())
    n = ch * W                      # spatial elems per partition per chunk
    mh = cfg.get("mh", 8)           # map super-chunk rows per half
    RWm = (mh + K - 1) * WP         # map cols per half per super-chunk
    piece = -(-RWm // -(-RWm // 512))           # matmul col piece (<=512)

    xp_d = nc.dram_tensor("xp", [CIN, HP * WP], dcv, kind="ExternalInput").ap()
    w_d = {}
    for nm in ("q", "k", "v"):
        for hb in "AB":
            w_d[nm + hb] = nc.dram_tensor(f"{nm}_w{hb}", [CIN, 128], dcv,
                                          kind="ExternalInput").ap()
    ea_d = nc.dram_tensor("emb_a", [C, W], f32, kind="ExternalInput").ap()
    eb_d = nc.dram_tensor("emb_b", [C, H], f32, kind="ExternalInput").ap()
    em_d = nc.dram_tensor("emb_mix", [C, H * W], dts["mix"], kind="ExternalInput").ap()
    out_d = nc.dram_tensor("out", [C, H * W], f32, kind="ExternalOutput").ap()

    with tile.TileContext(nc) as tc, ExitStack() as ctx:
        const = ctx.enter_context(tc.tile_pool(name="const", bufs=1))
        xp_p = ctx.enter_context(tc.tile_pool(name="xp", bufs=3))
        mix_p = ctx.enter_context(tc.tile_pool(name="mix", bufs=3))
        map_p = ctx.enter_context(tc.tile_pool(name="maps", bufs=2))
        kk_p = ctx.enter_context(tc.tile_pool(name="kk", bufs=2))
        kk3_p = ctx.enter_context(tc.tile_pool(name="kk3", bufs=3))
        sm_p = ctx.enter_context(tc.tile_pool(name="small", bufs=2))
        ps_kv = ctx.enter_context(tc.tile_pool(name="pskv", bufs=3, space="PSUM"))
        ps_q = ctx.enter_context(tc.tile_pool(name="psq", bufs=2, space="PSUM"))
        ps_acc = ctx.enter_context(tc.tile_pool(name="psacc", bufs=1, space="PSUM"))
        ps_acc2 = ctx.enter_context(tc.tile_pool(name="psacc2", bufs=1, space="PSUM"))

        # ---- constants ----
        w_t = {}
        for key, d in w_d.items():
            wtile = const.tile([CIN, 128], dcv, tag=f"w{key}")
            nc.sync.dma_start(wtile[:], d[:])
            w_t[key] = wtile
        ea_t = const.tile([128, W], f32, tag="ea")       # emb_a[c, w], both halves
        nc.sync.dma_start(ea_t[0:C, :], ea_d[:])
        nc.sync.dma_start(ea_t[C:128, :], ea_d[:])
        eb_t = const.tile([128, HH], f32, tag="eb")      # emb_b[c, 64*half + hl]
        nc.sync.dma_start(eb_t[0:C, :], _ap(eb_d, 0, [[1, HH]]))
        nc.sync.dma_start(eb_t[C:128, :], _ap(eb_d, HH, [[1, HH]]))
        ident = const.tile([128, 128], f32, tag="ident")
        masks.make_identity(nc, ident[:])
        idents = {f32: ident}
        for dt_ in {dts["e1"], d_e2, dts["m2"]} - {f32}:
            it = const.tile([128, 128], dt_, tag=f"ident{dt_}")
            nc.vector.tensor_copy(it[:], ident[:])
            idents[dt_] = it

        loop_cm = tc.For_i(0, reps, 1) if reps else _nullcm()
        with loop_cm:
            for mh0 in range(0, HH, mh):
                # ==== super-chunk: produce k/v/v^2 maps for mh rows per half ====
                xp_t = xp_p.tile([CIN, 2 * RWm], dcv, tag="xp")
                for half in (0, 1):
                    nc.sync.dma_start(
                        xp_t[:, half * RWm:(half + 1) * RWm],
                        _ap(xp_d, (HH * half + mh0) * WP, [[1, RWm]]))

                kv_ps = {}
                for name in ("k", "v"):
                    for pc in range(0, RWm, piece):
                        pw = min(piece, RWm - pc)
                        pt = ps_kv.tile([128, 512], f32, tag="kv")
                        kv_ps[(name, pc)] = (pt, pw)
                        for half, hb in ((0, "A"), (1, "B")):
                            nc.tensor.matmul(
                                pt[:, 0:pw], w_t[name + hb][:],
                                xp_t[:, half * RWm + pc: half * RWm + pc + pw],
                                start=(half == 0), stop=(half == 1))

                # PSUM -> SBUF maps (ACT); B variants shifted one element right so
                # odd-dj window reads stay 4-byte aligned for the DVE 2x mode.
                k_map = map_p.tile([128, RWm], dts["map"], tag="kmap")
                v_map = map_p.tile([128, RWm], dts["map"], tag="vmap")
                v2_map = map_p.tile([128, RWm], dts["map"], tag="v2map")
                for pc in range(0, RWm, piece):
                    pt, pw = kv_ps[("k", pc)]
                    nc.scalar.copy(k_map[:, pc:pc + pw], pt[:, 0:pw])
                    pt, pw = kv_ps[("v", pc)]
                    nc.scalar.copy(v_map[:, pc:pc + pw], pt[:, 0:pw])
                    nc.scalar.activation(v2_map[:, pc:pc + pw], pt[:, 0:pw], SQUARE)
                if split:
                    k_b = map_p.tile([128, RWm + 2], dts["map"], tag="kb")
                    v_b = map_p.tile([128, RWm + 2], dts["map"], tag="vb")
                    v2_b = map_p.tile([128, RWm + 2], dts["map"], tag="v2b")
                    for a_t, b_t in ((k_map, k_b), (v_map, v_b), (v2_map, v2_b)):
                        nc.gpsimd.tensor_copy(b_t[:, 1:RWm + 1], a_t[:, 0:RWm])
                else:
                    k_b = v_b = v2_b = None

                for h0 in range(mh0, mh0 + mh, ch):
                    ro = (h0 - mh0) * WP       # row offset into the map tiles
                    mix_t = mix_p.tile([128, n], dts["mix"], tag="mix")
                    for half in (0, 1):
                        nc.sync.dma_start(
                            mix_t[C * half:C * (half + 1), :],
                            _ap(em_d, (HH * half + h0) * W, [[1, n]]))

                    q_ps = ps_q.tile([128, 512], f32, tag="q")
                    for half, hb in ((0, "A"), (1, "B")):
                        rhs = _ap(xp_t[:],
                                  half * RWm + (h0 - mh0 + PAD) * WP + PAD,
                                  [[WP, ch], [1, W]])
                        nc.tensor.matmul(q_ps[:, 0:n], w_t["q" + hb][:],
                                         rhs, start=(half == 0), stop=(half == 1))

                    # ---- E = (emb_a + emb_b) * emb_mix  (on GPSIMD) ----
                    tmp_t = sm_p.tile([128, n], f32, tag="tmpE")
                    nc.gpsimd.tensor_tensor(
                        _ap(tmp_t[:], 0, [[W, ch], [1, W]]),
                        _ap(ea_t[:], 0, [[0, ch], [1, W]]),
                        _ap(eb_t[:], h0, [[1, ch], [0, W]]), ADD)
                    E_t = sm_p.tile([128, n], dts["E"], tag="E")
                    nc.gpsimd.tensor_tensor(E_t[:], tmp_t[:], mix_t[:], MULT)

                    # ---- KK-expanded stages ----
                    # ISA: max 3 free dims -> one instr per di (fp32), or per
                    # (di, dj-parity) when 16-bit (keeps every operand 4B-aligned).
                    def tt_kk(op_name, out_t, make_in0, make_in1):
                        if not split:
                            for di in range(K):
                                nc.vector.tensor_tensor(
                                    _ap(out_t[:], di * K * n,
                                        [[n, K], [W, ch], [1, W]]),
                                    make_in0(di, None), make_in1(di, None), MULT)
                        else:
                            for di in range(K):
                                eng = (nc.gpsimd if (op_name, di) in pool_tt
                                       else nc.vector)
                                for par in (0, 1):
                                    eng.tensor_tensor(
                                        _ap(out_t[:], (di * K + par) * n,
                                            [[2 * n, 2], [W, ch], [1, W]]),
                                        make_in0(di, par), make_in1(di, par), MULT)

                    def win_di(m_a, m_b):
                        def f(di, par):
                            if par is None:
                                return _ap(m_a[:], ro + di * WP,
                                           [[1, K], [WP, ch], [1, W]])
                            src = m_a if par == 0 else m_b
                            return _ap(src[:], ro + di * WP + 2 * par,
                                       [[2, 2], [WP, ch], [1, W]])
                        return f

                    def bc_c(c_t):
                        def f(di, par):
                            kdim = [0, K] if par is None else [0, 2]
                            return _ap(c_t[:], 0, [kdim, [W, ch], [1, W]])
                        return f

                    def kk_slice(k_t):
                        def f(di, par):
                            if par is None:
                                return _ap(k_t[:], di * K * n,
                                           [[n, K], [W, ch], [1, W]])
                            return _ap(k_t[:], (di * K + par) * n,
                                       [[2 * n, 2], [W, ch], [1, W]])
                        return f

                    def pe_reduce(src_t, acc_t, dt_):
                        # acc[(half,c), pos] = sum_kk src[(half,c), kk*n + pos]
                        # via 16 PSUM-accumulating identity matmuls (exact fp32).
                        for kk in range(KK):
                            nc.tensor.matmul(
                                acc_t[:, 0:n], idents[dt_][:],
                                src_t[:, kk * n:(kk + 1) * n],
                                start=(kk == 0), stop=(kk == KK - 1))

                    t1 = kk3_p.tile([128, KK * n], dts["t1"], tag="kkT1")
                    tt_kk("t1", t1, win_di(v2_map, v2_b), bc_c(E_t))
                    e1 = kk3_p.tile([128, KK * n], dts["e1"], tag="kkE1")
                    nc.scalar.activation(e1[:], t1[:], EXP)

                    r1_ps = ps_acc.tile([128, 512], f32, tag="r1")
                    pe_reduce(e1, r1_ps, dts["e1"])
                    rc1 = sm_p.tile([128, n], f32, tag="rc1")
                    nc.vector.reciprocal_approx_fast(rc1[:], r1_ps[:, 0:n])
                    qp_t = sm_p.tile([128, n], dts["qp"], tag="qp")
                    nc.vector.tensor_tensor(qp_t[:], q_ps[:, 0:n], rc1[:], MULT)

                    # f = q' * e1 is bounded by |q| -> fits fp16, and the
                    # contiguous x broadcast product needs no parity split.
                    # Then s2 = f * k_window (was m1 = k*e1; s2 = m1*q').
                    f_t = kk_p.tile([128, KK * n], dts["m1"], tag="kkM1")
                    nc.vector.tensor_tensor(
                        _ap(f_t[:], 0, [[n, KK], [W, ch], [1, W]]),
                        _ap(e1[:], 0, [[n, KK], [W, ch], [1, W]]),
                        _ap(qp_t[:], 0, [[0, KK], [W, ch], [1, W]]), MULT)
                    s2 = kk_p.tile([128, KK * n], dts["s2"], tag="kkS2")
                    tt_kk("s2", s2, win_di(k_map, k_b), kk_slice(f_t))
                    e2 = kk_p.tile([128, KK * n], d_e2, tag="kkE2")
                    nc.scalar.activation(e2[:], s2[:], EXP)

                    r2_ps = ps_acc.tile([128, 512], f32, tag="r2")
                    pe_reduce(e2, r2_ps, d_e2)
                    m2 = kk_p.tile([128, KK * n], dts["m2"], tag="kkM2")
                    if cfg["e2_fp32"]:
                        for di in range(K):
                            nc.vector.tensor_tensor(
                                _ap(m2[:], di * K * n, [[n, K], [W, ch], [1, W]]),
                                _ap(e2[:], di * K * n, [[n, K], [W, ch], [1, W]]),
                                win_di(v_map, v_b)(di, None), MULT)
                    else:
                        tt_kk("m2", m2, kk_slice(e2), win_di(v_map, v_b))
                    r3_ps = ps_acc2.tile([128, 512], f32, tag="r3")
                    pe_reduce(m2, r3_ps, dts["m2"])

                    rc2 = sm_p.tile([128, n], f32, tag="rc2")
                    nc.vector.reciprocal_approx_fast(rc2[:], r2_ps[:, 0:n])
                    out_t = sm_p.tile([128, n], f32, tag="out")
                    nc.vector.tensor_tensor(out_t[:], r3_ps[:, 0:n], rc2[:], MULT)

                    for half in (0, 1):
                        nc.sync.dma_start(
                            _ap(out_d, (HH * half + h0) * W, [[1, n]]),
                            out_t[C * half:C * (half + 1), :])


_compiled_nc = None


def _get_nc():
    global _compiled_nc
    if _compiled_nc is None:
        nc = bacc.Bacc("TRN2", target_bir_lowering=False, debug=False,
                       num_devices=N_CORES)
        build_kernel(nc)
        nc.compile()
        _compiled_nc = nc
    return _compiled_nc


def _shard_inputs(x, q_w, k_w, v_w, emb_a, emb_b, emb_mix):
    cv_np = mybir.dt.np(CFG["conv"])
    xp = np.pad(x.astype(np.float32), ((0, 0), (0, 0), (PAD, PAD), (PAD, PAD)))
    xp = xp.astype(cv_np)
    def padw(wT, hb):
        full = np.zeros((CIN, 128), np.float32)
        full[:, 64 * (hb == "B"):64 * (hb == "B") + C] = wT
        return np.ascontiguousarray(full.astype(cv_np))
    common = {
        "q_wA": padw(q_w.T, "A"), "q_wB": padw(q_w.T, "B"),
        "k_wA": padw(k_w.T, "A"), "k_wB": padw(k_w.T, "B"),
        "v_wA": padw(v_w.T, "A"), "v_wB": padw(v_w.T, "B"),
        "emb_a": np.ascontiguousarray(emb_a.astype(np.float32)),
        "emb_b": np.ascontiguousarray(emb_b.astype(np.float32)),
        "emb_mix": np.ascontiguousarray(emb_mix.reshape(C, H * W).astype(mybir.dt.np((CFG.get("dts") or {}).get("mix", CFG["kk"])))),
    }
    return [dict(common, xp=np.ascontiguousarray(xp[b].reshape(CIN, HP * WP)))
            for b in range(B)]


def kernel(x, q_w, k_w, v_w, emb_a, emb_b, emb_mix):
    x, q_w, k_w, v_w, emb_a, emb_b, emb_mix = (
        np.asarray(a, dtype=np.float32)
        for a in (x, q_w, k_w, v_w, emb_a, emb_b, emb_mix))
    nc = _get_nc()
    in_maps = _shard_inputs(x, q_w, k_w, v_w, emb_a, emb_b, emb_mix)
    res = bass_utils.run_bass_kernel_spmd(nc, in_maps, list(range(N_CORES)))
    out = np.stack([res.results[b]["out"].reshape(C, H, W) for b in range(B)])
    return out.astype(np.float32)

